# revision 44
# baseline (speedup 1.0000x reference)
"""Bilinear interpolation (dense warp) Trainium2 kernel, v2.

Data-parallel over batch (8 images/core x 8 cores).  Per 128-row tile the
sampled value is a separable 13-tap tent-weighted window sum

  out[r,c] = sum_m tenty_m(y) * Z_m[r,c]
  Z_m[r,c] = sum_k tentx_k(x) * I[r+m-6, c+k-6]     (tent = relu(1-|d|))

computed entirely in fp16 on the DVE (products, 2x mode) with PE
identity-matmul accumulation in PSUM.  The 13 row-shifted bands are DMA'd
straight from DRAM (edge rows/cols replicate-padded).  Reference border
semantics (trunc-toward-zero + clip with weights from clipped corners)
reduce to: linear extrapolation for x|y in (-1,0) -- folded into the tent
planes via diagonal-AP fixups -- and exact zero for x|y <= -1 or >= 511,
applied as a final threshold mask.
"""
import sys

sys.path.insert(0, "/opt/trn_rl_repo")
from contextlib import ExitStack

import numpy as np

from concourse import bass, mybir
import concourse.tile as tile
from concourse.bass_utils import run_bass_kernel_spmd
from concourse.vector_clock import ScopedClock
import bass_rust

# --- workaround: this walrus build rejects >2 sem waits on one instruction;
# TileContext's tail drain carries the whole global clock.  Redistribute.
def _patched_drain_and_barrier(self, tick_clock, wait_clock):
    drain_inst = self.nc.sync.drain()
    wait_clock.add_sem_waits(
        drain_inst.ins, ScopedClock({None: tick_clock.global_clock})
    )
    si = drain_inst.ins.sync_info
    if si is not None and si.on_wait and len(si.on_wait) > 1:
        waits = list(si.on_wait)
        si.on_wait = [waits[0]]
        sems = {h.name: h for h in self.sems.allocated().values()}
        for w in waits[1:]:
            h = sems.get(w.ant_name)
            assert h is not None, (w.ant_name, list(sems))
            assert w.wait_mode == "sem-ge-imm", w
            self.nc.sync.wait_ge(h, w.wait_value)
    self.nc.all_engine_barrier()
    assert self.sems is not None
    popped = self.nc._tile_sem_poison_stack.pop()
    assert popped is self._sem_poison
    self.nc.clear_and_free_semaphores(list(self.sems.allocated().values()))
    self.nc.all_engine_barrier()


tile.TileContext._drain_and_barrier = _patched_drain_and_barrier

# --- same walrus limit, general case: split any scheduled instruction that
# carries >1 sem wait into single-wait NoOps ahead of it.
_MAXW = 1
_nop_counter = [0]


def _split_multiwaits(ordered):
    for bb_name, insts in ordered.items():
        out = []
        changed = False
        for inst in insts:
            si = getattr(inst, "sync_info", None)
            if si is not None and si.on_wait and len(si.on_wait) > _MAXW:
                waits = list(si.on_wait)
                for w in waits[:-_MAXW]:
                    _nop_counter[0] += 1
                    nop = mybir.InstNoOp(
                        name=f"I-wsplit-{_nop_counter[0]}", ins=[], outs=[]
                    )
                    nop.engine = inst.engine
                    nop.sync_info = mybir.SyncInfo(on_wait=[w], on_update=[])
                    out.append(nop)
                si.on_wait = waits[-_MAXW:]
                changed = True
            out.append(inst)
        if changed:
            insts[:] = out


_orig_lower_ordered = tile.TileContext._lower_ordered_insts


def _patched_lower_ordered(self, ordered):
    _split_multiwaits(ordered)
    return _orig_lower_ordered(self, ordered)


tile.TileContext._lower_ordered_insts = _patched_lower_ordered

H = W = 512
IPC = 8  # images per core
NCORES = 8
PAD = 6
WPAD = W + 2 * PAD  # 524
NS = 13  # window taps (shift m-6, m = 0..12)
F16 = mybir.dt.float16
F32 = mybir.dt.float32
AL = mybir.AluOpType
AF = mybir.ActivationFunctionType

# const layout (element offsets within the [128, NCONST] f16 const tile)
O_THHI = 0          # [*, 512]  511 - c
O_THLO = 512        # [*, 512]  -1 - c
O_RM2 = 1024        # [*, 13*512]  y-extrap mask (+1 @ m=7-p, -1 @ m=6-p, p<6)
O_ID = 1024 + NS * W          # [*, 128] +identity
O_NEGID = O_ID + 128          # [*, 128] -identity
NCONST = O_NEGID + 128
# f32 per-partition consts (cstf): [:,t]=511-(r0_t+p); [:,4+t]=-1-(r0_t+p);
# [:,8+m]=6-m (abs bias); [:,21]=1.0 (relu bias)


def _ap(base_ap, dims, elem_offset):
    """Clone base_ap with custom free dims (strides/offset in elements)."""
    a = base_ap.copy()
    part = list(a.ap[0])
    a.ap = bass_rust.VecI64Pair([part] + [list(d) for d in dims])
    if elem_offset:
        a.offset = a.offset + elem_offset
    return a


def _do_tile(nc, pools, consts, img, t, dram, taps):
    imgs_d, dvx_d, dvy_d, out_d = dram
    CST, CSTF = consts
    (pl_dv, pl_sib, pl_scd, pl_tx, pl_ty, pl_prod, pl_zsb, pl_sc, pl_io,
     pl_psz, pl_pso) = pools
    r0 = 128 * t
    lox, hix, loy, hiy = taps
    TPX = hix - lox + 2  # x planes j: shift lox+j
    TPY = hiy - loy + 2  # y planes j: row r + loy + j

    DVX = pl_dv.tile([128, W], F16, tag="dvx", name="dvx")
    nc.sync.dma_start(out=DVX[:], in_=dvx_d[img, r0 : r0 + 128])
    DVY = pl_dv.tile([128, W], F16, tag="dvy", name="dvy")
    nc.sync.dma_start(out=DVY[:], in_=dvy_d[img, r0 : r0 + 128])

    SIBs = []
    for j in range(TPY):
        sib = pl_sib.tile([128, WPAD], F16, tag=f"sib{j}", name=f"sib{j}")
        base = r0 + loy + j  # image row of partition 0
        p0 = max(0, -base)
        p1 = min(128, H - base)
        nc.sync.dma_start(
            out=sib[p0:p1, :], in_=imgs_d[img, base + p0 : base + p1, :]
        )
        for p in range(0, p0):  # replicate row 0 (tile 0 edge)
            nc.sync.dma_start(out=sib[p : p + 1, :], in_=imgs_d[img, 0:1, :])
        for p in range(p1, 128):  # replicate row 511 (tile 3 edge)
            nc.sync.dma_start(
                out=sib[p : p + 1, :], in_=imgs_d[img, H - 1 : H, :]
            )
        SIBs.append(sib)

    # ---- masks: zero where x or y outside (-1, 511) ----------------------
    border = t in (0, 3)
    thhi = _ap(CST[:], [[1, W]], O_THHI)
    thlo = _ap(CST[:], [[1, W]], O_THLO)
    M = pl_sc.tile([128, W], F16, tag="mask", name="mask")
    M2 = pl_sc.tile([128, W], F16, tag="mask2", name="mask2")
    nc.vector.tensor_tensor(out=M[:], in0=DVX[:], in1=thhi, op=AL.is_lt)
    nc.vector.tensor_tensor(out=M2[:], in0=DVX[:], in1=thlo, op=AL.is_gt)
    nc.vector.tensor_tensor(out=M[:], in0=M[:], in1=M2[:], op=AL.mult)
    if border:
        thyhi = CSTF[:, t : t + 1]
        thylo = CSTF[:, 4 + t : 4 + t + 1]
        MY = pl_sc.tile([128, W], F16, tag="masky", name="masky")
        nc.vector.tensor_scalar(
            out=MY[:], in0=DVY[:], scalar1=thyhi, scalar2=None, op0=AL.is_lt
        )
        nc.vector.tensor_tensor(out=M[:], in0=M[:], in1=MY[:], op=AL.mult)
        nc.vector.tensor_scalar(
            out=MY[:], in0=DVY[:], scalar1=thylo, scalar2=None, op0=AL.is_gt
        )
        nc.vector.tensor_tensor(out=M[:], in0=M[:], in1=MY[:], op=AL.mult)

    # ---- X tent planes (positive: relu(1-|dx-(lox+j)|)), ScalarE --------
    # per-plane Abs into TX, then ONE wide in-place Relu: groups the ACT
    # table sets (Abs...Abs, Relu) and halves the ScalarE op count.
    TX = pl_tx.tile([128, NS * W], F16, tag="tx", name="tx")
    for j in range(TPX):
        bc = 8 + (lox + j + 6)  # CSTF col with value -(lox+j)
        nc.scalar.activation(
            out=TX[:, j * W : (j + 1) * W], in_=DVX[:], func=AF.Abs,
            bias=CSTF[:, bc : bc + 1], scale=1.0,
        )
    txw = TX[:, : TPX * W]
    nc.scalar.activation(
        out=txw, in_=txw, func=AF.Relu, bias=CSTF[:, 21:22], scale=-1.0
    )
    # x-extrap fix for x in (-1,0) at cols c < -lox: tent(col1) += minX,
    # tent(col0) -= minX.  col0 plane j = -c-lox.
    nfx = min(6, -lox)
    X6 = pl_sc.tile([128, 8], F16, tag="x6", name="x6")
    thlo6 = _ap(CST[:], [[1, nfx]], O_THLO)
    nc.vector.tensor_tensor(
        out=X6[:, 0:nfx], in0=DVX[:, 0:nfx], in1=thlo6, op=AL.subtract
    )
    nc.vector.tensor_scalar(
        out=X6[:, 0:nfx], in0=X6[:, 0:nfx], scalar1=-1.0, scalar2=0.0,
        op0=AL.add, op1=AL.min,
    )
    dhi = _ap(TX[:], [[-(W - 1), nfx]], (1 - lox) * W)
    dlo = _ap(TX[:], [[-(W - 1), nfx]], (-lox) * W)
    x6 = X6[:, 0:nfx]
    nc.vector.tensor_tensor(out=dhi, in0=dhi, in1=x6, op=AL.add)
    nc.vector.tensor_tensor(out=dlo, in0=dlo, in1=x6, op=AL.subtract)

    # ---- Y tent planes (positive: relu(1-|dy-(loy+j)|)), ScalarE --------
    TY = pl_ty.tile([128, NS * W], F16, tag="ty", name="ty")
    for j in range(TPY):
        bc = 8 + (loy + j + 6)
        nc.scalar.activation(
            out=TY[:, j * W : (j + 1) * W], in_=DVY[:], func=AF.Abs,
            bias=CSTF[:, bc : bc + 1], scale=1.0,
        )
    tyw = TY[:, : TPY * W]
    nc.scalar.activation(
        out=tyw, in_=tyw, func=AF.Relu, bias=CSTF[:, 21:22], scale=-1.0
    )
    if t == 0:
        # y-extrap for y in (-1,0) at rows 0..5: via RM2 const mask
        Yf = pl_sc.tile([128, W], F16, tag="yf", name="yf")
        prow = CSTF[:, 4:5]  # -1-p
        nc.vector.tensor_scalar(
            out=Yf[:], in0=DVY[:], scalar1=prow, scalar2=None, op0=AL.subtract
        )  # dy - (-1-p) = y + 1
        nc.vector.tensor_scalar(
            out=Yf[:], in0=Yf[:], scalar1=-1.0, scalar2=0.0,
            op0=AL.add, op1=AL.min,
        )  # min(y, 0)
        # fixes only target planes j = 1-p-loy and -p-loy for p<=5, so
        # j <= 1-loy: narrow the wide ops to nf planes
        nf = min(TPY, 2 - loy)
        rm2 = _ap(CST[:], [[W, nf], [1, W]], O_RM2 + (loy + 6) * W)
        ybc = _ap(Yf[:], [[0, nf], [1, W]], 0)
        FIX = pl_prod.tile([128, NS * W], F16, tag="prod", name="yfix")
        fx = _ap(FIX[:], [[W, nf], [1, W]], 0)
        nc.vector.tensor_tensor(out=fx, in0=rm2, in1=ybc, op=AL.mult)
        nc.vector.tensor_tensor(
            out=TY[:, : nf * W], in0=TY[:, : nf * W], in1=FIX[:, : nf * W],
            op=AL.add,
        )

    ident = _ap(CST[:], [[1, 128]], O_ID)
    negid = _ap(CST[:], [[1, 128]], O_NEGID)
    ZSB = pl_zsb.tile([128, NS * W], F16, tag="zsball", name="zsball")
    PRY = pl_zsb.tile([128, NS * W], F16, tag="prymega", name="prymega")

    # even/odd split by parity of the window offset 6+lox+j (fp16 pairs
    # must start 4B-aligned for the DVE 2x mode)
    jA0 = 0 if (6 + lox) % 2 == 0 else 1  # group A: even offsets, from sib
    jB0 = 1 - jA0                         # group B: odd offsets, from scd
    nA = (TPX - jA0 + 1) // 2
    nB = (TPX - jB0 + 1) // 2

    # ---- per-shift x-stage + y accumulation -----------------------------
    OUTP = pl_pso.tile([128, W], F32, tag="outp", name="outp")
    for j in range(TPY):
        sib = SIBs[j]
        scd = pl_scd.tile([128, WPAD - 1], F16, tag="scd", name="scd")
        nc.sync.dma_start(out=scd[:], in_=sib[:, 1:WPAD])
        prod = pl_prod.tile([128, NS * W], F16, tag="prod", name=f"prod{j}")
        CS = 396  # cols [0,CS) on DVE, [CS,512) on GPSIMD (idle engine)
        pa_out = _ap(prod[:], [[W, nA], [1, CS]], 0)
        pa_tx = _ap(TX[:], [[2 * W, nA], [1, CS]], jA0 * W)
        pa_sib = _ap(sib[:], [[2, nA], [1, CS]], 6 + lox + jA0)
        nc.vector.tensor_tensor(out=pa_out, in0=pa_tx, in1=pa_sib, op=AL.mult)
        ga_out = _ap(prod[:], [[W, nA], [1, W - CS]], CS)
        ga_tx = _ap(TX[:], [[2 * W, nA], [1, W - CS]], jA0 * W + CS)
        ga_sib = _ap(sib[:], [[2, nA], [1, W - CS]], 6 + lox + jA0 + CS)
        nc.gpsimd.tensor_tensor(out=ga_out, in0=ga_tx, in1=ga_sib, op=AL.mult)
        pb_out = _ap(prod[:], [[W, nB], [1, CS]], nA * W)
        pb_tx = _ap(TX[:], [[2 * W, nB], [1, CS]], jB0 * W)
        pb_scd = _ap(scd[:], [[2, nB], [1, CS]], 6 + lox + jB0 - 1)
        nc.vector.tensor_tensor(out=pb_out, in0=pb_tx, in1=pb_scd, op=AL.mult)
        gb_out = _ap(prod[:], [[W, nB], [1, W - CS]], nA * W + CS)
        gb_tx = _ap(TX[:], [[2 * W, nB], [1, W - CS]], jB0 * W + CS)
        gb_scd = _ap(scd[:], [[2, nB], [1, W - CS]], 6 + lox + jB0 - 1 + CS)
        nc.gpsimd.tensor_tensor(out=gb_out, in0=gb_tx, in1=gb_scd, op=AL.mult)

        ZP = pl_psz.tile([128, W], F32, tag="z", name=f"z{j}")
        for i in range(TPX):
            nc.tensor.matmul(
                ZP[:], lhsT=ident, rhs=prod[:, i * W : (i + 1) * W],
                start=(i == 0), stop=(i == TPX - 1), skip_group_check=True,
            )
        nc.scalar.copy(out=ZSB[:, j * W : (j + 1) * W], in_=ZP[:])
        # paired wide y-products: after odd j (or the last j), multiply the
        # accumulated zsb slice(s) by TY and feed the OUT accumulation
        if j % 2 == 1 or j == TPY - 1:
            jlo = j - 1 if j % 2 == 1 else j
            nw = (j - jlo + 1) * W
            nc.vector.tensor_tensor(
                out=PRY[:, jlo * W : jlo * W + nw],
                in0=TY[:, jlo * W : jlo * W + nw],
                in1=ZSB[:, jlo * W : jlo * W + nw], op=AL.mult,
            )
            for jj in range(jlo, j + 1):
                nc.tensor.matmul(
                    OUTP[:], lhsT=ident, rhs=PRY[:, jj * W : (jj + 1) * W],
                    start=(jj == 0), stop=(jj == TPY - 1),
                    skip_group_check=True,
                )

    OUTS = pl_io.tile([128, W], F32, tag="outs", name="outs")
    nc.vector.tensor_tensor(out=OUTS[:], in0=OUTP[:], in1=M[:], op=AL.mult)
    nc.sync.dma_start(out=out_d[img, r0 : r0 + 128, :], in_=OUTS[:])


def _host_consts():
    cst = np.zeros((128, NCONST), np.float16)
    c = np.arange(W, dtype=np.float32)
    p = np.arange(128, dtype=np.float32)
    cst[:, O_THHI : O_THHI + W] = (511.0 - c)[None, :]
    cst[:, O_THLO : O_THLO + W] = (-1.0 - c)[None, :]
    rm2 = np.zeros((128, NS, W), np.float32)
    for pp in range(6):
        rm2[pp, 7 - pp, :] = 1.0
        rm2[pp, 6 - pp, :] = -1.0
    cst[:, O_RM2 : O_RM2 + NS * W] = rm2.reshape(128, NS * W)
    ident = np.eye(128, dtype=np.float32)
    cst[:, O_ID : O_ID + 128] = ident
    cst[:, O_NEGID : O_NEGID + 128] = -ident
    cstf = np.zeros((128, 24), np.float32)
    for t in range(4):
        cstf[:, t] = 511.0 - (128 * t + p)
        cstf[:, 4 + t] = -1.0 - (128 * t + p)
    for m in range(NS):
        cstf[:, 8 + m] = 6.0 - m
    cstf[:, 21] = 1.0
    return cst, cstf


def _build(taps_tbl):
    nc = bass.Bass()
    imgs_d = nc.dram_tensor("imgs", [IPC, H, WPAD], F16, kind="ExternalInput").ap()
    dvx_d = nc.dram_tensor("dvx", [IPC, H, W], F16, kind="ExternalInput").ap()
    dvy_d = nc.dram_tensor("dvy", [IPC, H, W], F16, kind="ExternalInput").ap()
    cst_d = nc.dram_tensor("cst", [128, NCONST], F16, kind="ExternalInput").ap()
    cstf_d = nc.dram_tensor("cstf", [128, 24], F32, kind="ExternalInput").ap()
    out_d = nc.dram_tensor("out", [IPC, H, W], F32, kind="ExternalOutput").ap()
    dram = (imgs_d, dvx_d, dvy_d, out_d)

    with ExitStack() as ctx:
        tc = ctx.enter_context(tile.TileContext(nc))
        pl_const = ctx.enter_context(tc.tile_pool(name="const", bufs=1))
        pl_dv = ctx.enter_context(tc.tile_pool(name="dv", bufs=2))
        pl_sib = ctx.enter_context(tc.tile_pool(name="sib", bufs=2))
        pl_scd = ctx.enter_context(tc.tile_pool(name="scd", bufs=2))
        pl_tx = ctx.enter_context(tc.tile_pool(name="tx", bufs=2))
        pl_ty = ctx.enter_context(tc.tile_pool(name="ty", bufs=2))
        pl_prod = ctx.enter_context(tc.tile_pool(name="prod", bufs=3))
        pl_zsb = ctx.enter_context(tc.tile_pool(name="zsb", bufs=2))
        pl_sc = ctx.enter_context(tc.tile_pool(name="sc", bufs=2))
        pl_io = ctx.enter_context(tc.tile_pool(name="io", bufs=2))
        pl_psz = ctx.enter_context(tc.tile_pool(name="psz", bufs=5, space="PSUM"))
        pl_pso = ctx.enter_context(tc.tile_pool(name="pso", bufs=2, space="PSUM"))

        CST = pl_const.tile([128, NCONST], F16, name="cst")
        nc.sync.dma_start(out=CST[:], in_=cst_d[:, :])
        CSTF = pl_const.tile([128, 24], F32, name="cstf")
        nc.sync.dma_start(out=CSTF[:], in_=cstf_d[:, :])

        pools = (pl_dv, pl_sib, pl_scd, pl_tx, pl_ty, pl_prod, pl_zsb, pl_sc,
                 pl_io, pl_psz, pl_pso)
        for img in range(IPC):
            for t in range(4):
                _do_tile(nc, pools, (CST, CSTF), img, t, dram,
                         taps_tbl[img][t])
    return nc


_nc_cache = {}


def f16_trunc(a):
    # fp16 with round-toward-zero: the reference output is discontinuous at
    # the mask boundaries (x|y = -1 or 511, integer thresholds); truncation
    # keeps quantized coords on the same side of every boundary as the
    # original (nearest-rounding can land exactly on one and flip the side).
    b = np.ascontiguousarray(a, np.float32).view(np.uint32).copy()
    b &= np.uint32(0xFFFFE000)
    return b.view(np.float32).astype(np.float16)


def _plan(dvx, dvy):
    """Per-image-per-tile window bounds on the quantized field; sort images
    by window work so the worst windows share program slots."""
    B = dvx.shape[0]
    per = B // NCORES
    ntile = H // 128
    dx4 = dvx.reshape(B, ntile, 128, W).astype(np.float32)
    dy4 = dvy.reshape(B, ntile, 128, W).astype(np.float32)
    xlo = np.floor(dx4.min(axis=(2, 3))).astype(np.int64)
    xhi = np.floor(dx4.max(axis=(2, 3))).astype(np.int64)
    ylo = np.floor(dy4.min(axis=(2, 3))).astype(np.int64)
    yhi = np.floor(dy4.max(axis=(2, 3))).astype(np.int64)
    assert xlo.min() >= -PAD and xhi.max() <= PAD - 1, "displacement > pad"
    assert ylo.min() >= -PAD and yhi.max() <= PAD - 1, "displacement > pad"
    xlo = np.minimum(xlo, -1); ylo = np.minimum(ylo, -1)
    xhi = np.maximum(xhi, 0); yhi = np.maximum(yhi, 0)

    work = ((xhi - xlo + 2) * (yhi - ylo + 2)).sum(axis=1)
    order = np.argsort(-work, kind="stable")

    taps_tbl = []
    for j in range(per):
        idxs = order[j * NCORES : (j + 1) * NCORES]
        taps_tbl.append(tuple(
            (int(xlo[idxs, t].min()), int(xhi[idxs, t].max()),
             int(ylo[idxs, t].min()), int(yhi[idxs, t].max()))
            for t in range(ntile)
        ))
    return tuple(taps_tbl), order


def kernel(imgs: np.ndarray, dvfs: np.ndarray) -> np.ndarray:
    B = imgs.shape[0]
    assert imgs.shape == (B, H, W, 1) and dvfs.shape == (B, H, W, 2)
    per = B // NCORES

    im = imgs[..., 0].astype(np.float16)
    imp = np.empty((B, H, WPAD), np.float16)
    imp[:, :, PAD : PAD + W] = im
    imp[:, :, :PAD] = im[:, :, :1]
    imp[:, :, PAD + W :] = im[:, :, -1:]
    dvx = f16_trunc(dvfs[..., 0])
    dvy = f16_trunc(dvfs[..., 1])
    cst, cstf = _host_consts()
    taps_tbl, order = _plan(dvx, dvy)

    nc = _nc_cache.get(taps_tbl)
    if nc is None:
        nc = _nc_cache[taps_tbl] = _build(taps_tbl)

    in_maps = []
    for i in range(NCORES):
        idxs = order[np.arange(per) * NCORES + i]
        in_maps.append({
            "imgs": np.ascontiguousarray(imp[idxs]),
            "dvx": np.ascontiguousarray(dvx[idxs]),
            "dvy": np.ascontiguousarray(dvy[idxs]),
            "cst": cst,
            "cstf": cstf,
        })
    res = run_bass_kernel_spmd(nc, in_maps, list(range(NCORES)))
    global LAST_RESULT
    LAST_RESULT = res
    out = np.empty((B, H, W), np.float32)
    for i in range(NCORES):
        idxs = order[np.arange(per) * NCORES + i]
        out[idxs] = res.results[i]["out"]
    return out[..., None]


LAST_RESULT = None


# revision 48
# speedup vs baseline: 1.0825x; 1.0825x over previous
"""Bilinear interpolation (dense warp) Trainium2 kernel, v2.

Data-parallel over batch (8 images/core x 8 cores).  Per 128-row tile the
sampled value is a separable 13-tap tent-weighted window sum

  out[r,c] = sum_m tenty_m(y) * Z_m[r,c]
  Z_m[r,c] = sum_k tentx_k(x) * I[r+m-6, c+k-6]     (tent = relu(1-|d|))

computed entirely in fp16 on the DVE (products, 2x mode) with PE
identity-matmul accumulation in PSUM.  The 13 row-shifted bands are DMA'd
straight from DRAM (edge rows/cols replicate-padded).  Reference border
semantics (trunc-toward-zero + clip with weights from clipped corners)
reduce to: linear extrapolation for x|y in (-1,0) -- folded into the tent
planes via diagonal-AP fixups -- and exact zero for x|y <= -1 or >= 511,
applied as a final threshold mask.
"""
import sys

sys.path.insert(0, "/opt/trn_rl_repo")
from contextlib import ExitStack

import numpy as np

from concourse import bass, mybir
import concourse.tile as tile
from concourse.bass_utils import run_bass_kernel_spmd
from concourse.vector_clock import ScopedClock
import bass_rust

# --- workaround: this walrus build rejects >2 sem waits on one instruction;
# TileContext's tail drain carries the whole global clock.  Redistribute.
def _patched_drain_and_barrier(self, tick_clock, wait_clock):
    drain_inst = self.nc.sync.drain()
    wait_clock.add_sem_waits(
        drain_inst.ins, ScopedClock({None: tick_clock.global_clock})
    )
    si = drain_inst.ins.sync_info
    if si is not None and si.on_wait and len(si.on_wait) > 1:
        waits = list(si.on_wait)
        si.on_wait = [waits[0]]
        sems = {h.name: h for h in self.sems.allocated().values()}
        for w in waits[1:]:
            h = sems.get(w.ant_name)
            assert h is not None, (w.ant_name, list(sems))
            assert w.wait_mode == "sem-ge-imm", w
            self.nc.sync.wait_ge(h, w.wait_value)
    self.nc.all_engine_barrier()
    assert self.sems is not None
    popped = self.nc._tile_sem_poison_stack.pop()
    assert popped is self._sem_poison
    self.nc.clear_and_free_semaphores(list(self.sems.allocated().values()))
    self.nc.all_engine_barrier()


tile.TileContext._drain_and_barrier = _patched_drain_and_barrier

# --- same walrus limit, general case: split any scheduled instruction that
# carries >1 sem wait into single-wait NoOps ahead of it.
_MAXW = 1
_nop_counter = [0]


def _split_multiwaits(ordered):
    for bb_name, insts in ordered.items():
        out = []
        changed = False
        for inst in insts:
            si = getattr(inst, "sync_info", None)
            if si is not None and si.on_wait and len(si.on_wait) > _MAXW:
                waits = list(si.on_wait)
                for w in waits[:-_MAXW]:
                    _nop_counter[0] += 1
                    nop = mybir.InstNoOp(
                        name=f"I-wsplit-{_nop_counter[0]}", ins=[], outs=[]
                    )
                    nop.engine = inst.engine
                    nop.sync_info = mybir.SyncInfo(on_wait=[w], on_update=[])
                    out.append(nop)
                si.on_wait = waits[-_MAXW:]
                changed = True
            out.append(inst)
        if changed:
            insts[:] = out


_orig_lower_ordered = tile.TileContext._lower_ordered_insts


def _patched_lower_ordered(self, ordered):
    _split_multiwaits(ordered)
    return _orig_lower_ordered(self, ordered)


tile.TileContext._lower_ordered_insts = _patched_lower_ordered

H = W = 512
IPC = 8  # images per core
NCORES = 8
PAD = 6
WPAD = W + 2 * PAD  # 524
NS = 13  # window taps (shift m-6, m = 0..12)
F16 = mybir.dt.float16
F32 = mybir.dt.float32
AL = mybir.AluOpType
AF = mybir.ActivationFunctionType

# const layout (element offsets within the [128, NCONST] f16 const tile)
O_THHI = 0          # [*, 512]  511 - c
O_THLO = 512        # [*, 512]  -1 - c
O_RM2 = 1024        # [*, 13*512]  y-extrap mask (+1 @ m=7-p, -1 @ m=6-p, p<6)
O_ID = 1024 + NS * W          # [*, 128] +identity
O_NEGID = O_ID + 128          # [*, 128] -identity
NCONST = O_NEGID + 128
# f32 per-partition consts (cstf): [:,t]=511-(r0_t+p); [:,4+t]=-1-(r0_t+p);
# [:,8+m]=6-m (abs bias); [:,21]=1.0 (relu bias)


def _ap(base_ap, dims, elem_offset):
    """Clone base_ap with custom free dims (strides/offset in elements)."""
    a = base_ap.copy()
    part = list(a.ap[0])
    a.ap = bass_rust.VecI64Pair([part] + [list(d) for d in dims])
    if elem_offset:
        a.offset = a.offset + elem_offset
    return a


def _do_tile(nc, pools, consts, img, t, dram, taps):
    imgs_d, dvx_d, dvy_d, out_d = dram
    CST, CSTF = consts
    (pl_dv, pl_sib, pl_scd, pl_tx, pl_ty, pl_prod, pl_zsb, pl_sc, pl_io,
     pl_psz, pl_pso) = pools
    r0 = 128 * t
    lox, hix, loy, hiy = taps
    TPX = hix - lox + 2  # x planes j: shift lox+j
    TPY = hiy - loy + 2  # y planes j: row r + loy + j

    DVX = pl_dv.tile([128, W], F16, tag="dvx", name="dvx")
    nc.sync.dma_start(out=DVX[:], in_=dvx_d[img, r0 : r0 + 128])
    DVY = pl_dv.tile([128, W], F16, tag="dvy", name="dvy")
    nc.sync.dma_start(out=DVY[:], in_=dvy_d[img, r0 : r0 + 128])

    SIBs = []
    for j in range(TPY):
        sib = pl_sib.tile([128, WPAD], F16, tag=f"sib{j}", name=f"sib{j}")
        base = r0 + loy + j  # image row of partition 0
        p0 = max(0, -base)
        p1 = min(128, H - base)
        nc.sync.dma_start(
            out=sib[p0:p1, :], in_=imgs_d[img, base + p0 : base + p1, :]
        )
        # tile-0 head: partition p0-1 (image row -1) must replicate row 0
        # exactly (y in (-1,0) extrapolation); deeper rows only need finite
        # data (their tents are exactly 0), so bulk-fill with real rows.
        if p0 >= 1:
            nc.sync.dma_start(
                out=sib[p0 - 1 : p0, :], in_=imgs_d[img, 0:1, :]
            )
        if p0 >= 2:
            nc.sync.dma_start(
                out=sib[0 : p0 - 1, :], in_=imgs_d[img, 0 : p0 - 1, :]
            )
        # tile-3 tail: rows >= 512 are only touched by masked pixels --
        # any finite rows suffice
        if p1 < 128:
            nc.sync.dma_start(
                out=sib[p1:128, :], in_=imgs_d[img, H - (128 - p1) : H, :]
            )
        SIBs.append(sib)

    # ---- masks: zero where x or y outside (-1, 511) ----------------------
    border = t in (0, 3)
    thhi = _ap(CST[:], [[1, W]], O_THHI)
    thlo = _ap(CST[:], [[1, W]], O_THLO)
    M = pl_sc.tile([128, W], F16, tag="mask", name="mask")
    M2 = pl_sc.tile([128, W], F16, tag="mask2", name="mask2")
    nc.vector.tensor_tensor(out=M[:], in0=DVX[:], in1=thhi, op=AL.is_lt)
    nc.vector.tensor_tensor(out=M2[:], in0=DVX[:], in1=thlo, op=AL.is_gt)
    nc.vector.tensor_tensor(out=M[:], in0=M[:], in1=M2[:], op=AL.mult)
    if border:
        thyhi = CSTF[:, t : t + 1]
        thylo = CSTF[:, 4 + t : 4 + t + 1]
        MY = pl_sc.tile([128, W], F16, tag="masky", name="masky")
        nc.vector.tensor_scalar(
            out=MY[:], in0=DVY[:], scalar1=thyhi, scalar2=None, op0=AL.is_lt
        )
        nc.vector.tensor_tensor(out=M[:], in0=M[:], in1=MY[:], op=AL.mult)
        nc.vector.tensor_scalar(
            out=MY[:], in0=DVY[:], scalar1=thylo, scalar2=None, op0=AL.is_gt
        )
        nc.vector.tensor_tensor(out=M[:], in0=M[:], in1=MY[:], op=AL.mult)

    # ---- X tent planes (positive: relu(1-|dx-(lox+j)|)), ScalarE --------
    # per-plane Abs into TX, then ONE wide in-place Relu: groups the ACT
    # table sets (Abs...Abs, Relu) and halves the ScalarE op count.
    TX = pl_tx.tile([128, NS * W], F16, tag="tx", name="tx")
    for j in range(TPX):
        bc = 8 + (lox + j + 6)  # CSTF col with value -(lox+j)
        nc.scalar.activation(
            out=TX[:, j * W : (j + 1) * W], in_=DVX[:], func=AF.Abs,
            bias=CSTF[:, bc : bc + 1], scale=1.0,
        )
    txw = TX[:, : TPX * W]
    nc.scalar.activation(
        out=txw, in_=txw, func=AF.Relu, bias=CSTF[:, 21:22], scale=-1.0
    )
    # x-extrap fix for x in (-1,0) at cols c < -lox: tent(col1) += minX,
    # tent(col0) -= minX.  col0 plane j = -c-lox.
    nfx = min(6, -lox)
    X6 = pl_sc.tile([128, 8], F16, tag="x6", name="x6")
    thlo6 = _ap(CST[:], [[1, nfx]], O_THLO)
    nc.vector.tensor_tensor(
        out=X6[:, 0:nfx], in0=DVX[:, 0:nfx], in1=thlo6, op=AL.subtract
    )
    nc.vector.tensor_scalar(
        out=X6[:, 0:nfx], in0=X6[:, 0:nfx], scalar1=-1.0, scalar2=0.0,
        op0=AL.add, op1=AL.min,
    )
    dhi = _ap(TX[:], [[-(W - 1), nfx]], (1 - lox) * W)
    dlo = _ap(TX[:], [[-(W - 1), nfx]], (-lox) * W)
    x6 = X6[:, 0:nfx]
    nc.vector.tensor_tensor(out=dhi, in0=dhi, in1=x6, op=AL.add)
    nc.vector.tensor_tensor(out=dlo, in0=dlo, in1=x6, op=AL.subtract)

    # ---- Y tent planes (positive: relu(1-|dy-(loy+j)|)), ScalarE --------
    TY = pl_ty.tile([128, NS * W], F16, tag="ty", name="ty")
    for j in range(TPY):
        bc = 8 + (loy + j + 6)
        nc.scalar.activation(
            out=TY[:, j * W : (j + 1) * W], in_=DVY[:], func=AF.Abs,
            bias=CSTF[:, bc : bc + 1], scale=1.0,
        )
    tyw = TY[:, : TPY * W]
    nc.scalar.activation(
        out=tyw, in_=tyw, func=AF.Relu, bias=CSTF[:, 21:22], scale=-1.0
    )
    if t == 0:
        # y-extrap for y in (-1,0) at rows 0..5: via RM2 const mask
        Yf = pl_sc.tile([128, W], F16, tag="yf", name="yf")
        prow = CSTF[:, 4:5]  # -1-p
        nc.vector.tensor_scalar(
            out=Yf[:], in0=DVY[:], scalar1=prow, scalar2=None, op0=AL.subtract
        )  # dy - (-1-p) = y + 1
        nc.vector.tensor_scalar(
            out=Yf[:], in0=Yf[:], scalar1=-1.0, scalar2=0.0,
            op0=AL.add, op1=AL.min,
        )  # min(y, 0)
        # fixes only target planes j = 1-p-loy and -p-loy for p<=5, so
        # j <= 1-loy: narrow the wide ops to nf planes
        nf = min(TPY, 2 - loy)
        rm2 = _ap(CST[:], [[W, nf], [1, W]], O_RM2 + (loy + 6) * W)
        ybc = _ap(Yf[:], [[0, nf], [1, W]], 0)
        FIX = pl_prod.tile([128, NS * W], F16, tag="prod", name="yfix")
        fx = _ap(FIX[:], [[W, nf], [1, W]], 0)
        nc.vector.tensor_tensor(out=fx, in0=rm2, in1=ybc, op=AL.mult)
        nc.vector.tensor_tensor(
            out=TY[:, : nf * W], in0=TY[:, : nf * W], in1=FIX[:, : nf * W],
            op=AL.add,
        )

    ident = _ap(CST[:], [[1, 128]], O_ID)
    negid = _ap(CST[:], [[1, 128]], O_NEGID)
    ZSB = pl_zsb.tile([128, NS * W], F16, tag="zsball", name="zsball")
    PRY = pl_zsb.tile([128, NS * W], F16, tag="prymega", name="prymega")

    # even/odd split by parity of the window offset 6+lox+j (fp16 pairs
    # must start 4B-aligned for the DVE 2x mode)
    jA0 = 0 if (6 + lox) % 2 == 0 else 1  # group A: even offsets, from sib
    jB0 = 1 - jA0                         # group B: odd offsets, from scd
    nA = (TPX - jA0 + 1) // 2
    nB = (TPX - jB0 + 1) // 2

    # ---- per-shift x-stage + y accumulation -----------------------------
    OUTP = pl_pso.tile([128, W], F32, tag="outp", name="outp")
    for j in range(TPY):
        sib = SIBs[j]
        scd = pl_scd.tile([128, WPAD - 1], F16, tag="scd", name="scd")
        nc.sync.dma_start(out=scd[:], in_=sib[:, 1:WPAD])
        prod = pl_prod.tile([128, NS * W], F16, tag="prod", name=f"prod{j}")
        CS = 396  # cols [0,CS) on DVE, [CS,512) on GPSIMD (idle engine)
        pa_out = _ap(prod[:], [[W, nA], [1, CS]], 0)
        pa_tx = _ap(TX[:], [[2 * W, nA], [1, CS]], jA0 * W)
        pa_sib = _ap(sib[:], [[2, nA], [1, CS]], 6 + lox + jA0)
        nc.vector.tensor_tensor(out=pa_out, in0=pa_tx, in1=pa_sib, op=AL.mult)
        ga_out = _ap(prod[:], [[W, nA], [1, W - CS]], CS)
        ga_tx = _ap(TX[:], [[2 * W, nA], [1, W - CS]], jA0 * W + CS)
        ga_sib = _ap(sib[:], [[2, nA], [1, W - CS]], 6 + lox + jA0 + CS)
        nc.gpsimd.tensor_tensor(out=ga_out, in0=ga_tx, in1=ga_sib, op=AL.mult)
        pb_out = _ap(prod[:], [[W, nB], [1, CS]], nA * W)
        pb_tx = _ap(TX[:], [[2 * W, nB], [1, CS]], jB0 * W)
        pb_scd = _ap(scd[:], [[2, nB], [1, CS]], 6 + lox + jB0 - 1)
        nc.vector.tensor_tensor(out=pb_out, in0=pb_tx, in1=pb_scd, op=AL.mult)
        gb_out = _ap(prod[:], [[W, nB], [1, W - CS]], nA * W + CS)
        gb_tx = _ap(TX[:], [[2 * W, nB], [1, W - CS]], jB0 * W + CS)
        gb_scd = _ap(scd[:], [[2, nB], [1, W - CS]], 6 + lox + jB0 - 1 + CS)
        nc.gpsimd.tensor_tensor(out=gb_out, in0=gb_tx, in1=gb_scd, op=AL.mult)

        ZP = pl_psz.tile([128, W], F32, tag="z", name=f"z{j}")
        for i in range(TPX):
            nc.tensor.matmul(
                ZP[:], lhsT=ident, rhs=prod[:, i * W : (i + 1) * W],
                start=(i == 0), stop=(i == TPX - 1), skip_group_check=True,
            )
        nc.scalar.copy(out=ZSB[:, j * W : (j + 1) * W], in_=ZP[:])
        # paired wide y-products: after odd j (or the last j), multiply the
        # accumulated zsb slice(s) by TY and feed the OUT accumulation
        if j % 2 == 1 or j == TPY - 1:
            jlo = j - 1 if j % 2 == 1 else j
            nw = (j - jlo + 1) * W
            nc.vector.tensor_tensor(
                out=PRY[:, jlo * W : jlo * W + nw],
                in0=TY[:, jlo * W : jlo * W + nw],
                in1=ZSB[:, jlo * W : jlo * W + nw], op=AL.mult,
            )
            for jj in range(jlo, j + 1):
                nc.tensor.matmul(
                    OUTP[:], lhsT=ident, rhs=PRY[:, jj * W : (jj + 1) * W],
                    start=(jj == 0), stop=(jj == TPY - 1),
                    skip_group_check=True,
                )

    OUTS = pl_io.tile([128, W], F32, tag="outs", name="outs")
    nc.vector.tensor_tensor(out=OUTS[:], in0=OUTP[:], in1=M[:], op=AL.mult)
    nc.sync.dma_start(out=out_d[img, r0 : r0 + 128, :], in_=OUTS[:])


def _host_consts():
    cst = np.zeros((128, NCONST), np.float16)
    c = np.arange(W, dtype=np.float32)
    p = np.arange(128, dtype=np.float32)
    cst[:, O_THHI : O_THHI + W] = (511.0 - c)[None, :]
    cst[:, O_THLO : O_THLO + W] = (-1.0 - c)[None, :]
    rm2 = np.zeros((128, NS, W), np.float32)
    for pp in range(6):
        rm2[pp, 7 - pp, :] = 1.0
        rm2[pp, 6 - pp, :] = -1.0
    cst[:, O_RM2 : O_RM2 + NS * W] = rm2.reshape(128, NS * W)
    ident = np.eye(128, dtype=np.float32)
    cst[:, O_ID : O_ID + 128] = ident
    cst[:, O_NEGID : O_NEGID + 128] = -ident
    cstf = np.zeros((128, 24), np.float32)
    for t in range(4):
        cstf[:, t] = 511.0 - (128 * t + p)
        cstf[:, 4 + t] = -1.0 - (128 * t + p)
    for m in range(NS):
        cstf[:, 8 + m] = 6.0 - m
    cstf[:, 21] = 1.0
    return cst, cstf


def _build(taps_tbl):
    nc = bass.Bass()
    imgs_d = nc.dram_tensor("imgs", [IPC, H, WPAD], F16, kind="ExternalInput").ap()
    dvx_d = nc.dram_tensor("dvx", [IPC, H, W], F16, kind="ExternalInput").ap()
    dvy_d = nc.dram_tensor("dvy", [IPC, H, W], F16, kind="ExternalInput").ap()
    cst_d = nc.dram_tensor("cst", [128, NCONST], F16, kind="ExternalInput").ap()
    cstf_d = nc.dram_tensor("cstf", [128, 24], F32, kind="ExternalInput").ap()
    out_d = nc.dram_tensor("out", [IPC, H, W], F32, kind="ExternalOutput").ap()
    dram = (imgs_d, dvx_d, dvy_d, out_d)

    with ExitStack() as ctx:
        tc = ctx.enter_context(tile.TileContext(nc))
        pl_const = ctx.enter_context(tc.tile_pool(name="const", bufs=1))
        pl_dv = ctx.enter_context(tc.tile_pool(name="dv", bufs=2))
        pl_sib = ctx.enter_context(tc.tile_pool(name="sib", bufs=2))
        pl_scd = ctx.enter_context(tc.tile_pool(name="scd", bufs=2))
        pl_tx = ctx.enter_context(tc.tile_pool(name="tx", bufs=2))
        pl_ty = ctx.enter_context(tc.tile_pool(name="ty", bufs=2))
        pl_prod = ctx.enter_context(tc.tile_pool(name="prod", bufs=3))
        pl_zsb = ctx.enter_context(tc.tile_pool(name="zsb", bufs=2))
        pl_sc = ctx.enter_context(tc.tile_pool(name="sc", bufs=2))
        pl_io = ctx.enter_context(tc.tile_pool(name="io", bufs=2))
        pl_psz = ctx.enter_context(tc.tile_pool(name="psz", bufs=5, space="PSUM"))
        pl_pso = ctx.enter_context(tc.tile_pool(name="pso", bufs=2, space="PSUM"))

        CST = pl_const.tile([128, NCONST], F16, name="cst")
        nc.sync.dma_start(out=CST[:], in_=cst_d[:, :])
        CSTF = pl_const.tile([128, 24], F32, name="cstf")
        nc.sync.dma_start(out=CSTF[:], in_=cstf_d[:, :])

        pools = (pl_dv, pl_sib, pl_scd, pl_tx, pl_ty, pl_prod, pl_zsb, pl_sc,
                 pl_io, pl_psz, pl_pso)
        for img in range(IPC):
            for t in range(4):
                _do_tile(nc, pools, (CST, CSTF), img, t, dram,
                         taps_tbl[img][t])
    return nc


_nc_cache = {}


def f16_trunc(a):
    # fp16 with round-toward-zero: the reference output is discontinuous at
    # the mask boundaries (x|y = -1 or 511, integer thresholds); truncation
    # keeps quantized coords on the same side of every boundary as the
    # original (nearest-rounding can land exactly on one and flip the side).
    b = np.ascontiguousarray(a, np.float32).view(np.uint32).copy()
    b &= np.uint32(0xFFFFE000)
    return b.view(np.float32).astype(np.float16)


def _plan(dvx, dvy):
    """Per-image-per-tile window bounds on the quantized field; sort images
    by window work so the worst windows share program slots."""
    B = dvx.shape[0]
    per = B // NCORES
    ntile = H // 128
    dx4 = dvx.reshape(B, ntile, 128, W).astype(np.float32)
    dy4 = dvy.reshape(B, ntile, 128, W).astype(np.float32)
    xlo = np.floor(dx4.min(axis=(2, 3))).astype(np.int64)
    xhi = np.floor(dx4.max(axis=(2, 3))).astype(np.int64)
    ylo = np.floor(dy4.min(axis=(2, 3))).astype(np.int64)
    yhi = np.floor(dy4.max(axis=(2, 3))).astype(np.int64)
    assert xlo.min() >= -PAD and xhi.max() <= PAD - 1, "displacement > pad"
    assert ylo.min() >= -PAD and yhi.max() <= PAD - 1, "displacement > pad"
    xlo = np.minimum(xlo, -1); ylo = np.minimum(ylo, -1)
    xhi = np.maximum(xhi, 0); yhi = np.maximum(yhi, 0)

    work = ((xhi - xlo + 2) * (yhi - ylo + 2)).sum(axis=1)
    order = np.argsort(-work, kind="stable")

    taps_tbl = []
    for j in range(per):
        idxs = order[j * NCORES : (j + 1) * NCORES]
        taps_tbl.append(tuple(
            (int(xlo[idxs, t].min()), int(xhi[idxs, t].max()),
             int(ylo[idxs, t].min()), int(yhi[idxs, t].max()))
            for t in range(ntile)
        ))
    return tuple(taps_tbl), order


def kernel(imgs: np.ndarray, dvfs: np.ndarray) -> np.ndarray:
    B = imgs.shape[0]
    assert imgs.shape == (B, H, W, 1) and dvfs.shape == (B, H, W, 2)
    per = B // NCORES

    im = imgs[..., 0].astype(np.float16)
    imp = np.empty((B, H, WPAD), np.float16)
    imp[:, :, PAD : PAD + W] = im
    imp[:, :, :PAD] = im[:, :, :1]
    imp[:, :, PAD + W :] = im[:, :, -1:]
    dvx = f16_trunc(dvfs[..., 0])
    dvy = f16_trunc(dvfs[..., 1])
    cst, cstf = _host_consts()
    taps_tbl, order = _plan(dvx, dvy)

    nc = _nc_cache.get(taps_tbl)
    if nc is None:
        nc = _nc_cache[taps_tbl] = _build(taps_tbl)

    in_maps = []
    for i in range(NCORES):
        idxs = order[np.arange(per) * NCORES + i]
        in_maps.append({
            "imgs": np.ascontiguousarray(imp[idxs]),
            "dvx": np.ascontiguousarray(dvx[idxs]),
            "dvy": np.ascontiguousarray(dvy[idxs]),
            "cst": cst,
            "cstf": cstf,
        })
    res = run_bass_kernel_spmd(nc, in_maps, list(range(NCORES)))
    global LAST_RESULT
    LAST_RESULT = res
    out = np.empty((B, H, W), np.float32)
    for i in range(NCORES):
        idxs = order[np.arange(per) * NCORES + i]
        out[idxs] = res.results[i]["out"]
    return out[..., None]


LAST_RESULT = None


# revision 54
# speedup vs baseline: 1.2545x; 1.1589x over previous
"""Bilinear interpolation (dense warp) Trainium2 kernel, v2.

Data-parallel over batch (8 images/core x 8 cores).  Per 128-row tile the
sampled value is a separable 13-tap tent-weighted window sum

  out[r,c] = sum_m tenty_m(y) * Z_m[r,c]
  Z_m[r,c] = sum_k tentx_k(x) * I[r+m-6, c+k-6]     (tent = relu(1-|d|))

computed entirely in fp16 on the DVE (products, 2x mode) with PE
identity-matmul accumulation in PSUM.  The 13 row-shifted bands are DMA'd
straight from DRAM (edge rows/cols replicate-padded).  Reference border
semantics (trunc-toward-zero + clip with weights from clipped corners)
reduce to: linear extrapolation for x|y in (-1,0) -- folded into the tent
planes via diagonal-AP fixups -- and exact zero for x|y <= -1 or >= 511,
applied as a final threshold mask.
"""
import sys

sys.path.insert(0, "/opt/trn_rl_repo")
from contextlib import ExitStack

import numpy as np

from concourse import bass, mybir
import concourse.tile as tile
from concourse.bass_utils import run_bass_kernel_spmd
from concourse.vector_clock import ScopedClock
import bass_rust

# --- workaround: this walrus build rejects >2 sem waits on one instruction;
# TileContext's tail drain carries the whole global clock.  Redistribute.
def _patched_drain_and_barrier(self, tick_clock, wait_clock):
    drain_inst = self.nc.sync.drain()
    wait_clock.add_sem_waits(
        drain_inst.ins, ScopedClock({None: tick_clock.global_clock})
    )
    si = drain_inst.ins.sync_info
    if si is not None and si.on_wait and len(si.on_wait) > 1:
        waits = list(si.on_wait)
        si.on_wait = [waits[0]]
        sems = {h.name: h for h in self.sems.allocated().values()}
        for w in waits[1:]:
            h = sems.get(w.ant_name)
            assert h is not None, (w.ant_name, list(sems))
            assert w.wait_mode == "sem-ge-imm", w
            self.nc.sync.wait_ge(h, w.wait_value)
    self.nc.all_engine_barrier()
    assert self.sems is not None
    popped = self.nc._tile_sem_poison_stack.pop()
    assert popped is self._sem_poison
    self.nc.clear_and_free_semaphores(list(self.sems.allocated().values()))
    self.nc.all_engine_barrier()


tile.TileContext._drain_and_barrier = _patched_drain_and_barrier

# --- same walrus limit, general case: split any scheduled instruction that
# carries >1 sem wait into single-wait NoOps ahead of it.
_MAXW = 1
_nop_counter = [0]


def _split_multiwaits(ordered):
    for bb_name, insts in ordered.items():
        out = []
        changed = False
        for inst in insts:
            si = getattr(inst, "sync_info", None)
            if si is not None and si.on_wait and len(si.on_wait) > _MAXW:
                waits = list(si.on_wait)
                for w in waits[:-_MAXW]:
                    _nop_counter[0] += 1
                    nop = mybir.InstNoOp(
                        name=f"I-wsplit-{_nop_counter[0]}", ins=[], outs=[]
                    )
                    nop.engine = inst.engine
                    nop.sync_info = mybir.SyncInfo(on_wait=[w], on_update=[])
                    out.append(nop)
                si.on_wait = waits[-_MAXW:]
                changed = True
            out.append(inst)
        if changed:
            insts[:] = out


_orig_lower_ordered = tile.TileContext._lower_ordered_insts


def _patched_lower_ordered(self, ordered):
    _split_multiwaits(ordered)
    return _orig_lower_ordered(self, ordered)


tile.TileContext._lower_ordered_insts = _patched_lower_ordered

H = W = 512
IPC = 8  # images per core
NCORES = 8
PAD = 6
WPAD = W + 2 * PAD  # 524
NS = 13  # window taps (shift m-6, m = 0..12)
F16 = mybir.dt.float16
F32 = mybir.dt.float32
AL = mybir.AluOpType
AF = mybir.ActivationFunctionType

# const layout (element offsets within the [128, NCONST] f16 const tile)
O_THHI = 0          # [*, 512]  511 - c
O_THLO = 512        # [*, 512]  -1 - c
O_RM2 = 1024        # [*, 13*512]  y-extrap mask (+1 @ m=7-p, -1 @ m=6-p, p<6)
O_ID = 1024 + NS * W          # [*, 128] +identity
O_NEGID = O_ID + 128          # [*, 128] -identity
NCONST = O_NEGID + 128
# f32 per-partition consts (cstf): [:,t]=511-(r0_t+p); [:,4+t]=-1-(r0_t+p);
# [:,8+m]=6-m (abs bias); [:,21]=1.0 (relu bias)


def _ap(base_ap, dims, elem_offset):
    """Clone base_ap with custom free dims (strides/offset in elements)."""
    a = base_ap.copy()
    part = list(a.ap[0])
    a.ap = bass_rust.VecI64Pair([part] + [list(d) for d in dims])
    if elem_offset:
        a.offset = a.offset + elem_offset
    return a


def _do_tile(nc, pools, consts, img, t, dram, taps):
    imgs_d, dvx_d, dvy_d, out_d = dram
    CST, CSTF = consts
    (pl_dv, pl_sib, pl_scd, pl_tx, pl_ty, pl_prod, pl_zsb, pl_sc, pl_io,
     pl_psz, pl_pso) = pools
    r0 = 128 * t
    lox, hix, loy, hiy = taps
    TPX = hix - lox + 2  # x planes j: shift lox+j
    TPY = hiy - loy + 2  # y planes j: row r + loy + j

    DVX = pl_dv.tile([128, W], F16, tag="dvx", name="dvx")
    nc.sync.dma_start(out=DVX[:], in_=dvx_d[img, r0 : r0 + 128])
    DVY = pl_dv.tile([128, W], F16, tag="dvy", name="dvy")
    nc.sync.dma_start(out=DVY[:], in_=dvy_d[img, r0 : r0 + 128])

    SIBs = []
    for j in range(TPY):
        sib = pl_sib.tile([128, WPAD], F16, tag=f"sib{j}", name=f"sib{j}")
        base = r0 + loy + j  # image row of partition 0
        p0 = max(0, -base)
        p1 = min(128, H - base)
        nc.sync.dma_start(
            out=sib[p0:p1, :], in_=imgs_d[img, base + p0 : base + p1, :]
        )
        # tile-0 head: partition p0-1 (image row -1) must replicate row 0
        # exactly (y in (-1,0) extrapolation); deeper rows only need finite
        # data (their tents are exactly 0), so bulk-fill with real rows.
        if p0 >= 1:
            nc.sync.dma_start(
                out=sib[p0 - 1 : p0, :], in_=imgs_d[img, 0:1, :]
            )
        if p0 >= 2:
            nc.sync.dma_start(
                out=sib[0 : p0 - 1, :], in_=imgs_d[img, 0 : p0 - 1, :]
            )
        # tile-3 tail: rows >= 512 are only touched by masked pixels --
        # any finite rows suffice
        if p1 < 128:
            nc.sync.dma_start(
                out=sib[p1:128, :], in_=imgs_d[img, H - (128 - p1) : H, :]
            )
        SIBs.append(sib)

    # ---- masks: zero where x or y outside (-1, 511) ----------------------
    border = t in (0, 3)
    thhi = _ap(CST[:], [[1, W]], O_THHI)
    thlo = _ap(CST[:], [[1, W]], O_THLO)
    M = pl_sc.tile([128, W], F16, tag="mask", name="mask")
    M2 = pl_sc.tile([128, W], F16, tag="mask2", name="mask2")
    nc.vector.tensor_tensor(out=M[:], in0=DVX[:], in1=thhi, op=AL.is_lt)
    nc.vector.tensor_tensor(out=M2[:], in0=DVX[:], in1=thlo, op=AL.is_gt)
    nc.vector.tensor_tensor(out=M[:], in0=M[:], in1=M2[:], op=AL.mult)
    if border:
        thyhi = CSTF[:, t : t + 1]
        thylo = CSTF[:, 4 + t : 4 + t + 1]
        MY = pl_sc.tile([128, W], F16, tag="masky", name="masky")
        nc.vector.tensor_scalar(
            out=MY[:], in0=DVY[:], scalar1=thyhi, scalar2=None, op0=AL.is_lt
        )
        nc.vector.tensor_tensor(out=M[:], in0=M[:], in1=MY[:], op=AL.mult)
        nc.vector.tensor_scalar(
            out=MY[:], in0=DVY[:], scalar1=thylo, scalar2=None, op0=AL.is_gt
        )
        nc.vector.tensor_tensor(out=M[:], in0=M[:], in1=MY[:], op=AL.mult)

    # ---- X tent planes (positive: relu(1-|dx-(lox+j)|)), ScalarE --------
    # per-plane Abs into TX, then ONE wide in-place Relu: groups the ACT
    # table sets (Abs...Abs, Relu) and halves the ScalarE op count.
    TX = pl_tx.tile([128, NS * W], F16, tag="tx", name="tx")
    for j in range(TPX):
        bc = 8 + (lox + j + 6)  # CSTF col with value -(lox+j)
        nc.scalar.activation(
            out=TX[:, j * W : (j + 1) * W], in_=DVX[:], func=AF.Abs,
            bias=CSTF[:, bc : bc + 1], scale=1.0,
        )
    txw = TX[:, : TPX * W]
    nc.scalar.activation(
        out=txw, in_=txw, func=AF.Relu, bias=CSTF[:, 21:22], scale=-1.0
    )
    # x-extrap fix for x in (-1,0) at cols c < -lox: tent(col1) += minX,
    # tent(col0) -= minX.  col0 plane j = -c-lox.
    nfx = min(6, -lox)
    X6 = pl_sc.tile([128, 8], F16, tag="x6", name="x6")
    thlo6 = _ap(CST[:], [[1, nfx]], O_THLO)
    nc.vector.tensor_tensor(
        out=X6[:, 0:nfx], in0=DVX[:, 0:nfx], in1=thlo6, op=AL.subtract
    )
    nc.vector.tensor_scalar(
        out=X6[:, 0:nfx], in0=X6[:, 0:nfx], scalar1=-1.0, scalar2=0.0,
        op0=AL.add, op1=AL.min,
    )
    dhi = _ap(TX[:], [[-(W - 1), nfx]], (1 - lox) * W)
    dlo = _ap(TX[:], [[-(W - 1), nfx]], (-lox) * W)
    x6 = X6[:, 0:nfx]
    nc.vector.tensor_tensor(out=dhi, in0=dhi, in1=x6, op=AL.add)
    nc.vector.tensor_tensor(out=dlo, in0=dlo, in1=x6, op=AL.subtract)

    # ---- Y tent planes (positive: relu(1-|dy-(loy+j)|)), ScalarE --------
    TY = pl_ty.tile([128, NS * W], F16, tag="ty", name="ty")
    for j in range(TPY):
        bc = 8 + (loy + j + 6)
        nc.scalar.activation(
            out=TY[:, j * W : (j + 1) * W], in_=DVY[:], func=AF.Abs,
            bias=CSTF[:, bc : bc + 1], scale=1.0,
        )
    tyw = TY[:, : TPY * W]
    nc.scalar.activation(
        out=tyw, in_=tyw, func=AF.Relu, bias=CSTF[:, 21:22], scale=-1.0
    )
    if t == 0:
        # y-extrap for y in (-1,0) at rows 0..5: via RM2 const mask
        Yf = pl_sc.tile([128, W], F16, tag="yf", name="yf")
        prow = CSTF[:, 4:5]  # -1-p
        nc.vector.tensor_scalar(
            out=Yf[:], in0=DVY[:], scalar1=prow, scalar2=None, op0=AL.subtract
        )  # dy - (-1-p) = y + 1
        nc.vector.tensor_scalar(
            out=Yf[:], in0=Yf[:], scalar1=-1.0, scalar2=0.0,
            op0=AL.add, op1=AL.min,
        )  # min(y, 0)
        # fixes only target planes j = 1-p-loy and -p-loy for p<=5, so
        # j <= 1-loy: narrow the wide ops to nf planes
        nf = min(TPY, 2 - loy)
        rm2 = _ap(CST[:], [[W, nf], [1, W]], O_RM2 + (loy + 6) * W)
        ybc = _ap(Yf[:], [[0, nf], [1, W]], 0)
        FIX = pl_prod.tile([128, NS * W], F16, tag="prod", name="yfix")
        fx = _ap(FIX[:], [[W, nf], [1, W]], 0)
        nc.vector.tensor_tensor(out=fx, in0=rm2, in1=ybc, op=AL.mult)
        nc.vector.tensor_tensor(
            out=TY[:, : nf * W], in0=TY[:, : nf * W], in1=FIX[:, : nf * W],
            op=AL.add,
        )

    ident = _ap(CST[:], [[1, 128]], O_ID)
    negid = _ap(CST[:], [[1, 128]], O_NEGID)
    ZSB = pl_zsb.tile([128, NS * W], F16, tag="zsball", name="zsball")
    PRY = pl_zsb.tile([128, NS * W], F16, tag="prymega", name="prymega")

    # even/odd split by parity of the window offset 6+lox+j (fp16 pairs
    # must start 4B-aligned for the DVE 2x mode)
    jA0 = 0 if (6 + lox) % 2 == 0 else 1  # group A: even offsets, from sib
    jB0 = 1 - jA0                         # group B: odd offsets, from scd
    nA = (TPX - jA0 + 1) // 2
    nB = (TPX - jB0 + 1) // 2

    # ---- per-shift x-stage + y accumulation -----------------------------
    OUTP = pl_pso.tile([128, W], F32, tag="outp", name="outp")
    for j in range(TPY):
        sib = SIBs[j]
        scd = pl_scd.tile([128, WPAD - 1], F16, tag="scd", name="scd")
        nc.sync.dma_start(out=scd[:], in_=sib[:, 1:WPAD])
        prod = pl_prod.tile([128, NS * W], F16, tag="prod", name=f"prod{j}")
        CS = 396  # cols [0,CS) on DVE, [CS,512) on GPSIMD (idle engine)
        pa_out = _ap(prod[:], [[W, nA], [1, CS]], 0)
        pa_tx = _ap(TX[:], [[2 * W, nA], [1, CS]], jA0 * W)
        pa_sib = _ap(sib[:], [[2, nA], [1, CS]], 6 + lox + jA0)
        nc.vector.tensor_tensor(out=pa_out, in0=pa_tx, in1=pa_sib, op=AL.mult)
        ga_out = _ap(prod[:], [[W, nA], [1, W - CS]], CS)
        ga_tx = _ap(TX[:], [[2 * W, nA], [1, W - CS]], jA0 * W + CS)
        ga_sib = _ap(sib[:], [[2, nA], [1, W - CS]], 6 + lox + jA0 + CS)
        nc.gpsimd.tensor_tensor(out=ga_out, in0=ga_tx, in1=ga_sib, op=AL.mult)
        pb_out = _ap(prod[:], [[W, nB], [1, CS]], nA * W)
        pb_tx = _ap(TX[:], [[2 * W, nB], [1, CS]], jB0 * W)
        pb_scd = _ap(scd[:], [[2, nB], [1, CS]], 6 + lox + jB0 - 1)
        nc.vector.tensor_tensor(out=pb_out, in0=pb_tx, in1=pb_scd, op=AL.mult)
        gb_out = _ap(prod[:], [[W, nB], [1, W - CS]], nA * W + CS)
        gb_tx = _ap(TX[:], [[2 * W, nB], [1, W - CS]], jB0 * W + CS)
        gb_scd = _ap(scd[:], [[2, nB], [1, W - CS]], 6 + lox + jB0 - 1 + CS)
        nc.gpsimd.tensor_tensor(out=gb_out, in0=gb_tx, in1=gb_scd, op=AL.mult)

        ZP = pl_psz.tile([128, W], F32, tag="z", name=f"z{j}")
        for i in range(TPX):
            nc.tensor.matmul(
                ZP[:], lhsT=ident, rhs=prod[:, i * W : (i + 1) * W],
                start=(i == 0), stop=(i == TPX - 1), skip_group_check=True,
            )
        nc.scalar.copy(out=ZSB[:, j * W : (j + 1) * W], in_=ZP[:])
        # paired wide y-products: after odd j (or the last j), multiply the
        # accumulated zsb slice(s) by TY and feed the OUT accumulation
        if j % 2 == 1 or j == TPY - 1:
            jlo = j - 1 if j % 2 == 1 else j
            nw = (j - jlo + 1) * W
            nc.vector.tensor_tensor(
                out=PRY[:, jlo * W : jlo * W + nw],
                in0=TY[:, jlo * W : jlo * W + nw],
                in1=ZSB[:, jlo * W : jlo * W + nw], op=AL.mult,
            )
            for jj in range(jlo, j + 1):
                nc.tensor.matmul(
                    OUTP[:], lhsT=ident, rhs=PRY[:, jj * W : (jj + 1) * W],
                    start=(jj == 0), stop=(jj == TPY - 1),
                    skip_group_check=True,
                )

    OUTS = pl_io.tile([128, W], F32, tag="outs", name="outs")
    nc.vector.tensor_tensor(out=OUTS[:], in0=OUTP[:], in1=M[:], op=AL.mult)
    nc.sync.dma_start(out=out_d[img, r0 : r0 + 128, :], in_=OUTS[:])


def _host_consts():
    cst = np.zeros((128, NCONST), np.float16)
    c = np.arange(W, dtype=np.float32)
    p = np.arange(128, dtype=np.float32)
    cst[:, O_THHI : O_THHI + W] = (511.0 - c)[None, :]
    cst[:, O_THLO : O_THLO + W] = (-1.0 - c)[None, :]
    rm2 = np.zeros((128, NS, W), np.float32)
    for pp in range(6):
        rm2[pp, 7 - pp, :] = 1.0
        rm2[pp, 6 - pp, :] = -1.0
    cst[:, O_RM2 : O_RM2 + NS * W] = rm2.reshape(128, NS * W)
    ident = np.eye(128, dtype=np.float32)
    cst[:, O_ID : O_ID + 128] = ident
    cst[:, O_NEGID : O_NEGID + 128] = -ident
    cstf = np.zeros((128, 24), np.float32)
    for t in range(4):
        cstf[:, t] = 511.0 - (128 * t + p)
        cstf[:, 4 + t] = -1.0 - (128 * t + p)
    for m in range(NS):
        cstf[:, 8 + m] = 6.0 - m
    cstf[:, 21] = 1.0
    return cst, cstf


def _build(taps_tbl):
    nc = bass.Bass()
    imgs_d = nc.dram_tensor("imgs", [IPC, H, WPAD], F16, kind="ExternalInput").ap()
    dvx_d = nc.dram_tensor("dvx", [IPC, H, W], F16, kind="ExternalInput").ap()
    dvy_d = nc.dram_tensor("dvy", [IPC, H, W], F16, kind="ExternalInput").ap()
    cst_d = nc.dram_tensor("cst", [128, NCONST], F16, kind="ExternalInput").ap()
    cstf_d = nc.dram_tensor("cstf", [128, 24], F32, kind="ExternalInput").ap()
    out_d = nc.dram_tensor("out", [IPC, H, W], F32, kind="ExternalOutput").ap()
    dram = (imgs_d, dvx_d, dvy_d, out_d)

    with ExitStack() as ctx:
        tc = ctx.enter_context(tile.TileContext(nc))
        pl_const = ctx.enter_context(tc.tile_pool(name="const", bufs=1))
        pl_dv = ctx.enter_context(tc.tile_pool(name="dv", bufs=3))
        pl_sib = ctx.enter_context(tc.tile_pool(name="sib", bufs=2))
        pl_scd = ctx.enter_context(tc.tile_pool(name="scd", bufs=3))
        pl_tx = ctx.enter_context(tc.tile_pool(name="tx", bufs=2))
        pl_ty = ctx.enter_context(tc.tile_pool(name="ty", bufs=2))
        pl_prod = ctx.enter_context(tc.tile_pool(name="prod", bufs=3))
        pl_zsb = ctx.enter_context(tc.tile_pool(name="zsb", bufs=2))
        pl_sc = ctx.enter_context(tc.tile_pool(name="sc", bufs=2))
        pl_io = ctx.enter_context(tc.tile_pool(name="io", bufs=2))
        pl_psz = ctx.enter_context(tc.tile_pool(name="psz", bufs=5, space="PSUM"))
        pl_pso = ctx.enter_context(tc.tile_pool(name="pso", bufs=2, space="PSUM"))

        CST = pl_const.tile([128, NCONST], F16, name="cst")
        nc.sync.dma_start(out=CST[:], in_=cst_d[:, :])
        CSTF = pl_const.tile([128, 24], F32, name="cstf")
        nc.sync.dma_start(out=CSTF[:], in_=cstf_d[:, :])

        pools = (pl_dv, pl_sib, pl_scd, pl_tx, pl_ty, pl_prod, pl_zsb, pl_sc,
                 pl_io, pl_psz, pl_pso)
        for img in range(IPC):
            for t in range(4):
                _do_tile(nc, pools, (CST, CSTF), img, t, dram,
                         taps_tbl[img][t])
    return nc


_nc_cache = {}


def f16_trunc(a):
    # fp16 with round-toward-zero: the reference output is discontinuous at
    # the mask boundaries (x|y = -1 or 511, integer thresholds); truncation
    # keeps quantized coords on the same side of every boundary as the
    # original (nearest-rounding can land exactly on one and flip the side).
    b = np.ascontiguousarray(a, np.float32).view(np.uint32).copy()
    b &= np.uint32(0xFFFFE000)
    return b.view(np.float32).astype(np.float16)


def _plan(dvx, dvy):
    """Per-image-per-tile window bounds on the quantized field; sort images
    by window work so the worst windows share program slots."""
    B = dvx.shape[0]
    per = B // NCORES
    ntile = H // 128
    dx4 = dvx.reshape(B, ntile, 128, W).astype(np.float32)
    dy4 = dvy.reshape(B, ntile, 128, W).astype(np.float32)
    xlo = np.floor(dx4.min(axis=(2, 3))).astype(np.int64)
    xhi = np.floor(dx4.max(axis=(2, 3))).astype(np.int64)
    ylo = np.floor(dy4.min(axis=(2, 3))).astype(np.int64)
    yhi = np.floor(dy4.max(axis=(2, 3))).astype(np.int64)
    assert xlo.min() >= -PAD and xhi.max() <= PAD - 1, "displacement > pad"
    assert ylo.min() >= -PAD and yhi.max() <= PAD - 1, "displacement > pad"
    xlo = np.minimum(xlo, -1); ylo = np.minimum(ylo, -1)
    xhi = np.maximum(xhi, 0); yhi = np.maximum(yhi, 0)

    work = ((xhi - xlo + 2) * (yhi - ylo + 2)).sum(axis=1)
    order = np.argsort(-work, kind="stable")

    taps_tbl = []
    for j in range(per):
        idxs = order[j * NCORES : (j + 1) * NCORES]
        taps_tbl.append(tuple(
            (int(xlo[idxs, t].min()), int(xhi[idxs, t].max()),
             int(ylo[idxs, t].min()), int(yhi[idxs, t].max()))
            for t in range(ntile)
        ))
    return tuple(taps_tbl), order


def kernel(imgs: np.ndarray, dvfs: np.ndarray) -> np.ndarray:
    B = imgs.shape[0]
    assert imgs.shape == (B, H, W, 1) and dvfs.shape == (B, H, W, 2)
    per = B // NCORES

    im = imgs[..., 0].astype(np.float16)
    imp = np.empty((B, H, WPAD), np.float16)
    imp[:, :, PAD : PAD + W] = im
    imp[:, :, :PAD] = im[:, :, :1]
    imp[:, :, PAD + W :] = im[:, :, -1:]
    dvx = f16_trunc(dvfs[..., 0])
    dvy = f16_trunc(dvfs[..., 1])
    cst, cstf = _host_consts()
    taps_tbl, order = _plan(dvx, dvy)

    nc = _nc_cache.get(taps_tbl)
    if nc is None:
        nc = _nc_cache[taps_tbl] = _build(taps_tbl)

    in_maps = []
    for i in range(NCORES):
        idxs = order[np.arange(per) * NCORES + i]
        in_maps.append({
            "imgs": np.ascontiguousarray(imp[idxs]),
            "dvx": np.ascontiguousarray(dvx[idxs]),
            "dvy": np.ascontiguousarray(dvy[idxs]),
            "cst": cst,
            "cstf": cstf,
        })
    res = run_bass_kernel_spmd(nc, in_maps, list(range(NCORES)))
    global LAST_RESULT
    LAST_RESULT = res
    out = np.empty((B, H, W), np.float32)
    for i in range(NCORES):
        idxs = order[np.arange(per) * NCORES + i]
        out[idxs] = res.results[i]["out"]
    return out[..., None]


LAST_RESULT = None
